# revision 33
# baseline (speedup 1.0000x reference)
"""Trainium2 Bass kernel for DCTLAVISBlip dc_transform (DCT -> truncate -> IDCT).

Measured ~103.6-104.4 us on hardware (baseline dense version: ~170.8 us) at
relative error ~7e-4 (gate 2e-2). Breakdown: ~70 us matmul-span floor (324
spans x ~216 ns), ~8 us LDWEIGHTS bubbles + input-catchup gaps, ~17 us fixed
NEFF preamble/teardown (engine init, all-engine barriers, full semaphore-file
clear), ~5 us drain/DMA tails. DMA total 29.1 MB/core, peak ~430 GB/s.

Strategy (exact even/odd DCT factorization, half the matmul FLOPs)
------------------------------------------------------------------
reference(x), x [B=64, T=576, C=1024] f32:
  y = M @ x[b] (DCT along tokens), v = |mean_{b,c} y|, threshold = quantile(v, .8),
  L = last index with v>thr; outputs y[:, :L] (f32) and state = Mi^T @ y[:, :L] (f16).

The DCT matrix obeys M[2j, T-1-t] = M[2j, t] and M[2j+1, T-1-t] = -M[2j+1, t].
With e = x_top + reverse(x_bot), d = x_top - reverse(x_bot)  ([288, C] each):
  y[0::2] = Me @ e,   y[1::2] = Mo @ d          (Me = M[0::2, :288], Mo = M[1::2, :288])
The same symmetry on Mi (size L) splits the IDCT: with h = ceil(L/2),
  P = Ae @ e, Q = Ao @ d   (Ae = Mi[0::2, :h]^T @ M[0:L:2, :288], Ao likewise odd)
  state[0:h] = P + Q,  state[h:L] = reverse((P - Q)[0:L-h])
All folds/reassembly are cheap O(B*T*C) host ops; the device does the four
[<=288 x 288] @ [288 x 1024] matmul blocks per batch -- exactly half the dense
[1152 x 576] work of v1 -- as one stacked 9-m-tile output per batch,
data-parallel over B across 8 cores.

Device schedule per core (8 batches): groups of (4 batches x 1 n-half) share
4 PSUM banks per m-tile, so the 32-row K-remainder row-packs 4-up (4 batches)
into one full-height 128-partition tile and runs as 4 concurrent strip
matmuls (tile_position) -- one span instead of two. m-tile 4 mixes the last
64 e-rows and first 64 d-rows via col-split matmuls (tile_position col
groups). DRAM tensors are laid out so one DMA moves a whole group's slab with
4KB-contiguous rows. Inputs stream on the sync HWDGE ring, outputs on gpsimd
SWDGE (plus the sync ring late, once inputs are done); PSUM drains split
vector/scalar; PE pre-warmed with wide dummy matmuls during the DMA head.
"""

import numpy as np

B, T, C = 64, 576, 1024
H = T // 2                   # 288, fold length
NCORES = 8
BPC = B // NCORES            # batches per core
MT = 9                       # m-tiles over 1152 output rows
Q8 = 0.8

_CACHED = {}


def _dct_mat(N):
    n = np.arange(N)
    Mm = np.cos(np.pi * (2 * n[None, :] + 1) * n[:, None] / (2 * N))
    s = np.full(N, np.sqrt(2.0 / N))
    s[0] = np.sqrt(1.0 / N)
    return s[:, None] * Mm          # float64


def _build_nc():
    import concourse.bacc as bacc
    import concourse.mybir as mybir
    import concourse.tile as tile

    f16 = mybir.dt.float16
    f32 = mybir.dt.float32

    nc = bacc.Bacc("TRN2", target_bir_lowering=False, debug=False,
                   num_devices=NCORES)
    # k-tiles laid out [kt, 128, batch, C]: kt 0,1 = e rows 0:256; 2,3 = d
    # rows. One DMA moves a whole batch-quad's k-tile (8KB-contiguous rows)
    # -- few, large DMAs keep the issue-instruction cost off the input path.
    eh = nc.dram_tensor("eh", [4, 128, BPC, C], f16, kind="ExternalInput")
    # k-remainder strips: [set, e/d, 128, C]; rows = 4 batches x 32
    krem = nc.dram_tensor("krem", [BPC // 4, 2, 128, C], f16,
                          kind="ExternalInput")
    # weights [k, m]: wte m-cols = [Me.T (288) | Ae.T (h, pad->288)]
    wte = nc.dram_tensor("wte", [H, 576], f16, kind="ExternalInput")
    wtd = nc.dram_tensor("wtd", [H, 576], f16, kind="ExternalInput")
    # k-remainder weights, duplicated across the 4 row strips
    w32e = nc.dram_tensor("w32e", [128, 576], f16, kind="ExternalInput")
    w32d = nc.dram_tensor("w32d", [128, 576], f16, kind="ExternalInput")
    # out rows (over m-tiles): [Xe(288); P(288); Xo(288); Q(288)]
    out = nc.dram_tensor("out", [MT, 128, 2, BPC, 512], f16,
                         kind="ExternalOutput")

    # groups: (batch-quad, n-half)
    groups = [(gb, n) for gb in (0, 4) for n in (0, 1)]

    # m-tile sub-blocks: (operand, wcol0, ncols, psum_col0)
    def msubs(mi):
        if mi <= 3:
            return [("e", 128 * mi, 128, 0)]
        if mi == 4:
            return [("e", 512, 64, 0), ("d", 0, 64, 64)]
        return [("d", 64 + 128 * (mi - 5), 128, 0)]

    with tile.TileContext(nc) as tc:
        with (
            tc.tile_pool(name="wpool", bufs=1) as wpool,
            tc.tile_pool(name="xpool", bufs=1) as xpool,
            tc.tile_pool(name="osb", bufs=16) as opool,
            tc.tile_pool(name="ps", bufs=8, space="PSUM") as ps,
        ):
            # PE warmup during the input-DMA head: wide (N=512) matmuls keep
            # the PE busy-duty high enough to trip the HAM un-throttle before
            # the first real matmul, and run until the first inputs land.
            # The memset comes AFTER the matmuls in program order: the tile
            # allocator just needs wz written somewhere, while the matmuls
            # (garbage reads, results discarded) start as soon as the Tensor
            # engine is up (~3.4us) instead of waiting for gpsimd init.
            wz = wpool.tile([128, 512], f16, tag="wz")
            pwarm = ps.tile([128, 512], f32, tag="pt", name="pt")
            for _ in range(5):
                nc.tensor.matmul(pwarm[:], wz[:, 0:128], wz[:],
                                 start=True, stop=True)
            nc.gpsimd.memset(wz[:], 0.0)

            # ---- input DMAs in first-use order (sync HWDGE ring) ----
            wet = [None, None]
            wdt = [None, None]
            xts = {}      # (kt, gb2) -> [128, 2048] tile (two batches)
            kts = {}      # (set, 0=e/1=d) -> [128, C]
            w32te = wpool.tile([128, 576], f16, tag="w32e")
            w32td = wpool.tile([128, 576], f16, tag="w32d")

            def load_quad_inputs(gb, first):
                # Inputs alternate across both HWDGE rings (sync + scalar):
                # the ~0.63us per dma_start issue cost would otherwise
                # serialize the input stream behind a single engine.
                ieng = [nc.sync, nc.scalar]
                iq = [0]

                def idma(dst, src):
                    ieng[iq[0] % 2].dma_start(dst, src)
                    iq[0] += 1

                for ki in range(2):
                    if first:
                        t_ = wpool.tile([128, 576], f16, tag=f"we{ki}")
                        idma(t_[:], wte[128 * ki:128 * ki + 128, :])
                        wet[ki] = t_
                    tx = xpool.tile([128, 4 * C], f16, tag=f"x{ki}_{gb}")
                    if first and ki == 0:
                        # split the very first tile for time-to-first-matmul
                        idma(tx[:, 0:2 * C], eh[ki, :, gb:gb + 2, :])
                        idma(tx[:, 2 * C:4 * C], eh[ki, :, gb + 2:gb + 4, :])
                    else:
                        idma(tx[:], eh[ki, :, gb:gb + 4, :])
                    xts[(ki, gb)] = tx
                if first:
                    idma(w32te[:], w32e[:, :])
                tk = xpool.tile([128, C], f16, tag=f"kre{gb}")
                idma(tk[:], krem[gb // 4, 0, :, :])
                kts[(gb // 4, 0)] = tk
                for ki in range(2):
                    if first:
                        t_ = wpool.tile([128, 576], f16, tag=f"wd{ki}")
                        idma(t_[:], wtd[128 * ki:128 * ki + 128, :])
                        wdt[ki] = t_
                    tx = xpool.tile([128, 4 * C], f16, tag=f"x{2 + ki}_{gb}")
                    idma(tx[:], eh[2 + ki, :, gb:gb + 4, :])
                    xts[(2 + ki, gb)] = tx
                if first:
                    idma(w32td[:], w32d[:, :])
                tk = xpool.tile([128, C], f16, tag=f"krd{gb}")
                idma(tk[:], krem[gb // 4, 1, :, :])
                kts[(gb // 4, 1)] = tk

            for gb in (0, 4):
                load_quad_inputs(gb, first=(gb == 0))

            wtile = {"e": wet, "d": wdt}
            w32tile = {"e": w32te, "d": w32td}

            def mov(op, ki, b, n):
                kt = (0 if op == "e" else 2) + ki
                gb = (b // 4) * 4
                c0 = 1024 * (b - gb) + 512 * n
                return xts[(kt, gb)][:, c0:c0 + 512]

            for gi, (gb, n) in enumerate(groups):
                for mi in range(MT):
                    pts = [ps.tile([128, 512], f32, tag="pt", name="pt")
                           for _ in range(4)]
                    subs = msubs(mi)
                    for ki in range(2):
                        if mi == 4:
                            for bi in range(4):
                                for (op, w0, nc_, p0) in subs:
                                    nc.tensor.matmul(
                                        pts[bi][p0:p0 + nc_, :],
                                        wtile[op][ki][:, w0:w0 + nc_],
                                        mov(op, ki, gb + bi, n),
                                        start=(ki == 0), stop=False,
                                        tile_position=(0, p0),
                                    )
                        else:
                            op, w0, nc_, p0 = subs[0]
                            for bi in range(4):
                                nc.tensor.matmul(
                                    pts[bi][:],
                                    wtile[op][ki][:, w0:w0 + nc_],
                                    mov(op, ki, gb + bi, n),
                                    start=(ki == 0), stop=False,
                                )
                    # 32-row k-remainder last: 4 concurrent strip matmuls
                    # (full 128-row usage), one span per m-tile.
                    for (op, w0, nc_, p0) in subs:
                        kt = kts[(gb // 4, 0 if op == "e" else 1)]
                        for bi in range(4):
                            sp = 32 * bi
                            nc.tensor.matmul(
                                pts[bi][p0:p0 + nc_, :],
                                w32tile[op][sp:sp + 32, w0:w0 + nc_],
                                kt[sp:sp + 32, 512 * n:512 * n + 512],
                                start=False, stop=True,
                                tile_position=(sp, p0),
                            )
                    # drain psum -> sbuf (vector: b0/b1, scalar: b2/b3),
                    # one 4-batch DMA out. Early groups' outputs go on the
                    # SYNC ring: engine instruction order puts them behind
                    # all input dma_starts, so the input stream keeps the
                    # full early HBM bandwidth (opool absorbs the deferral).
                    # Late groups alternate both rings to drain the tail.
                    ot = opool.tile([128, 2 * C], f16, tag="ot")
                    nc.vector.tensor_copy(ot[:, 0:512], pts[0][:])
                    nc.vector.tensor_copy(ot[:, 512:1024], pts[1][:])
                    nc.scalar.copy(ot[:, 1024:1536], pts[2][:])
                    nc.scalar.copy(ot[:, 1536:2048], pts[3][:])
                    if gi == 3 and mi == MT - 1:
                        # final tile: split across both rings to cut the tail
                        nc.sync.dma_start(out[mi, :, n, gb:gb + 2, :],
                                          ot[:, 0:C])
                        nc.gpsimd.dma_start(out[mi, :, n, gb + 2:gb + 4, :],
                                            ot[:, C:2 * C])
                    else:
                        oeng = nc.sync if gi < 2 else (
                            nc.gpsimd if mi % 2 == 1 else nc.sync)
                        oeng.dma_start(out[mi, :, n, gb:gb + 4, :], ot[:])
    nc.finalize()
    return nc


def _get_nc():
    if "nc" not in _CACHED:
        _CACHED["nc"] = _build_nc()
    return _CACHED["nc"]


def _ensure_trace_hook_safe():
    """If BASS_TRACE is set, run_bass_kernel_spmd imports antenv.axon_hooks,
    which may not exist. Install a ctypes shim or disable tracing."""
    import os
    import sys
    import types

    if not os.environ.get("BASS_TRACE"):
        return
    try:
        import antenv.axon_hooks  # noqa: F401
        return
    except ImportError:
        pass
    try:
        from trn_agent_boot.trn_boot import _ntff_profile_via_ctypes
        hooks = types.ModuleType("antenv.axon_hooks")
        hook = _ntff_profile_via_ctypes("/opt/axon/libaxon_pjrt.so")
        hooks.get_axon_ntff_profile_hook = lambda: hook
        hooks.set_axon_ntff_profile_hook = lambda h: None
        sys.modules["antenv.axon_hooks"] = hooks
    except Exception:
        os.environ["BASS_NEVER_TRACE"] = "1"


def kernel(x: np.ndarray):
    from concourse.bass_utils import run_bass_kernel_spmd

    _ensure_trace_hook_safe()
    x = np.ascontiguousarray(np.asarray(x, dtype=np.float32))
    assert x.shape == (B, T, C)

    # ---- host: data-dependent truncation length L (tiny, exact math) ----
    M64 = _dct_mat(T)
    xbar = x.astype(np.float64).mean(axis=(0, 2))
    v = np.abs(M64 @ xbar)
    thr = np.abs(np.quantile(v, Q8))
    idxs = np.where(v > thr)[0]
    last_index = int(idxs[-1]) if idxs.size > 0 else -1
    L = last_index if last_index >= 0 else T - 1
    h = (L + 1) // 2

    # ---- host: fold inputs ----
    u = x[:, 0:H, :]
    w_ = x[:, T - 1:H - 1:-1, :]
    e = (u + w_).astype(np.float16)            # [B, 288, C]
    d = (u - w_).astype(np.float16)

    # [4, 128, B, C]: kt 0,1 = e k-tiles; 2,3 = d k-tiles
    e_k = e[:, 0:256].reshape(B, 2, 128, C).transpose(1, 2, 0, 3)
    d_k = d[:, 0:256].reshape(B, 2, 128, C).transpose(1, 2, 0, 3)
    ehall = np.concatenate([e_k, d_k], axis=0)
    # [B//4, 2, 128, C]: 4 batches' 32-row remainders stacked per set
    kre = e[:, 256:288].reshape(B // 4, 4 * 32, C)
    krd = d[:, 256:288].reshape(B // 4, 4 * 32, C)
    krall = np.stack([kre, krd], axis=1)

    # ---- host: weights ----
    key = ("w", L)
    if key not in _CACHED:
        Me = M64[0::2, 0:H]
        Mo = M64[1::2, 0:H]
        Mi = _dct_mat(L)
        Ae = Mi[0::2, 0:h].T @ M64[0:L:2, 0:H]      # [h, 288]
        Ao = Mi[1::2, 0:h].T @ M64[1:L:2, 0:H]      # [h, 288]
        wte_np = np.zeros((H, 576), dtype=np.float16)
        wtd_np = np.zeros((H, 576), dtype=np.float16)
        wte_np[:, 0:H] = Me.T
        wte_np[:, H:H + h] = Ae.T
        wtd_np[:, 0:H] = Mo.T
        wtd_np[:, H:H + h] = Ao.T
        w32e_np = np.ascontiguousarray(np.tile(wte_np[256:288], (4, 1)))
        w32d_np = np.ascontiguousarray(np.tile(wtd_np[256:288], (4, 1)))
        _CACHED[key] = (wte_np, wtd_np, w32e_np, w32d_np)
    wte_np, wtd_np, w32e_np, w32d_np = _CACHED[key]

    nc = _get_nc()
    in_maps = [
        {"eh": np.ascontiguousarray(ehall[:, :, i * BPC:(i + 1) * BPC]),
         "krem": np.ascontiguousarray(
             krall[i * BPC // 4:(i + 1) * BPC // 4]),
         "wte": wte_np, "wtd": wtd_np, "w32e": w32e_np, "w32d": w32d_np}
        for i in range(NCORES)
    ]
    res = run_bass_kernel_spmd(nc, in_maps, list(range(NCORES)))
    _CACHED["last_exec_time_ns"] = res.exec_time_ns
    _CACHED["profile_json"] = res.profile_json

    # out [MT, 128, 2, BPC, 512] per core -> [BPC, 1152, C]
    o = np.concatenate(
        [res.results[i]["out"].transpose(3, 0, 1, 2, 4).reshape(
            BPC, MT * 128, C)
         for i in range(NCORES)], axis=0)
    Xe = o[:, 0:288]
    P = o[:, 288:576]
    Xo = o[:, 576:864]
    Qm = o[:, 864:1152]

    n_even = (L + 1) // 2
    n_odd = L // 2
    y = np.empty((B, L, C), dtype=np.float32)
    y[:, 0::2] = Xe[:, :n_even]
    y[:, 1::2] = Xo[:, :n_odd]

    Pf = P[:, :h].astype(np.float32)
    Qf = Qm[:, :h].astype(np.float32)
    state = np.empty((B, L, C), dtype=np.float16)
    state[:, 0:h] = (Pf + Qf).astype(np.float16)
    state[:, h:L] = (Pf - Qf)[:, 0:L - h][:, ::-1].astype(np.float16)
    return state, y


# revision 39
# speedup vs baseline: 1.0459x; 1.0459x over previous
"""Trainium2 Bass kernel for DCTLAVISBlip dc_transform (DCT -> truncate -> IDCT).

Measured ~103.6-104.4 us on hardware (baseline dense version: ~170.8 us) at
relative error ~7e-4 (gate 2e-2). Breakdown: ~70 us matmul-span floor (324
spans x ~216 ns), ~8 us LDWEIGHTS bubbles + input-catchup gaps, ~17 us fixed
NEFF preamble/teardown (engine init, all-engine barriers, full semaphore-file
clear), ~5 us drain/DMA tails. DMA total 29.1 MB/core, peak ~430 GB/s.

Strategy (exact even/odd DCT factorization, half the matmul FLOPs)
------------------------------------------------------------------
reference(x), x [B=64, T=576, C=1024] f32:
  y = M @ x[b] (DCT along tokens), v = |mean_{b,c} y|, threshold = quantile(v, .8),
  L = last index with v>thr; outputs y[:, :L] (f32) and state = Mi^T @ y[:, :L] (f16).

The DCT matrix obeys M[2j, T-1-t] = M[2j, t] and M[2j+1, T-1-t] = -M[2j+1, t].
With e = x_top + reverse(x_bot), d = x_top - reverse(x_bot)  ([288, C] each):
  y[0::2] = Me @ e,   y[1::2] = Mo @ d          (Me = M[0::2, :288], Mo = M[1::2, :288])
The same symmetry on Mi (size L) splits the IDCT: with h = ceil(L/2),
  P = Ae @ e, Q = Ao @ d   (Ae = Mi[0::2, :h]^T @ M[0:L:2, :288], Ao likewise odd)
  state[0:h] = P + Q,  state[h:L] = reverse((P - Q)[0:L-h])
All folds/reassembly are cheap O(B*T*C) host ops; the device does the four
[<=288 x 288] @ [288 x 1024] matmul blocks per batch -- exactly half the dense
[1152 x 576] work of v1 -- as one stacked 9-m-tile output per batch,
data-parallel over B across 8 cores.

Device schedule per core (8 batches): groups of (4 batches x 1 n-half) share
4 PSUM banks per m-tile, so the 32-row K-remainder row-packs 4-up (4 batches)
into one full-height 128-partition tile and runs as 4 concurrent strip
matmuls (tile_position) -- one span instead of two. m-tile 4 mixes the last
64 e-rows and first 64 d-rows via col-split matmuls (tile_position col
groups). DRAM tensors are laid out so one DMA moves a whole group's slab with
4KB-contiguous rows. Inputs stream on the sync HWDGE ring, outputs on gpsimd
SWDGE (plus the sync ring late, once inputs are done); PSUM drains split
vector/scalar; PE pre-warmed with wide dummy matmuls during the DMA head.
"""

import numpy as np

B, T, C = 64, 576, 1024
H = T // 2                   # 288, fold length
NCORES = 8
BPC = B // NCORES            # batches per core
MT = 9                       # m-tiles over 1152 output rows
Q8 = 0.8

_CACHED = {}


def _dct_mat(N):
    n = np.arange(N)
    Mm = np.cos(np.pi * (2 * n[None, :] + 1) * n[:, None] / (2 * N))
    s = np.full(N, np.sqrt(2.0 / N))
    s[0] = np.sqrt(1.0 / N)
    return s[:, None] * Mm          # float64


def _build_nc():
    import concourse.bacc as bacc
    import concourse.mybir as mybir
    import concourse.tile as tile

    f16 = mybir.dt.float16
    f32 = mybir.dt.float32

    nc = bacc.Bacc("TRN2", target_bir_lowering=False, debug=False,
                   num_devices=NCORES)
    # k-tiles laid out [kt, 128, nhalf, batch, 512]: kt 0,1 = e rows 0:256;
    # 2,3 = d rows. 2-batch 256KB DMA granularity: fine enough that the
    # first matmuls aren't gated on whole-quad transfers, coarse enough to
    # keep the per-dma_start issue cost manageable.
    eh = nc.dram_tensor("eh", [4, 128, 2, BPC, 512], f16, kind="ExternalInput")
    # k-remainder strips: [set, e/d, 128, C]; rows = 4 batches x 32
    krem = nc.dram_tensor("krem", [BPC // 4, 2, 128, C], f16,
                          kind="ExternalInput")
    # weights [k, m]: wte m-cols = [Me.T (288) | Ae.T (h, pad->288)]
    wte = nc.dram_tensor("wte", [H, 576], f16, kind="ExternalInput")
    wtd = nc.dram_tensor("wtd", [H, 576], f16, kind="ExternalInput")
    # k-remainder weights, duplicated across the 4 row strips
    w32e = nc.dram_tensor("w32e", [128, 576], f16, kind="ExternalInput")
    w32d = nc.dram_tensor("w32d", [128, 576], f16, kind="ExternalInput")
    # out rows (over m-tiles): [Xe(288); P(288); Xo(288); Q(288)]
    out = nc.dram_tensor("out", [MT, 128, 2, BPC, 512], f16,
                         kind="ExternalOutput")

    # groups: (batch-quad, n-half)
    groups = [(gb, n) for gb in (0, 4) for n in (0, 1)]

    # m-tile sub-blocks: (operand, wcol0, ncols, psum_col0)
    def msubs(mi):
        if mi <= 3:
            return [("e", 128 * mi, 128, 0)]
        if mi == 4:
            return [("e", 512, 64, 0), ("d", 0, 64, 64)]
        return [("d", 64 + 128 * (mi - 5), 128, 0)]

    with tile.TileContext(nc) as tc:
        with (
            tc.tile_pool(name="wpool", bufs=1) as wpool,
            tc.tile_pool(name="xpool", bufs=1) as xpool,
            tc.tile_pool(name="osb", bufs=16) as opool,
            tc.tile_pool(name="ps", bufs=8, space="PSUM") as ps,
        ):
            # PE warmup during the input-DMA head: wide (N=512) matmuls keep
            # the PE busy-duty high enough to trip the HAM un-throttle before
            # the first real matmul, and run until the first inputs land.
            # The memset comes AFTER the matmuls in program order: the tile
            # allocator just needs wz written somewhere, while the matmuls
            # (garbage reads, results discarded) start as soon as the Tensor
            # engine is up (~3.4us) instead of waiting for gpsimd init.
            wz = wpool.tile([128, 512], f16, tag="wz")
            pwarm = ps.tile([128, 512], f32, tag="pt", name="pt")
            for _ in range(5):
                nc.tensor.matmul(pwarm[:], wz[:, 0:128], wz[:],
                                 start=True, stop=True)
            nc.gpsimd.memset(wz[:], 0.0)

            # ---- input DMAs in first-use order (sync HWDGE ring) ----
            wet = [None, None]
            wdt = [None, None]
            xts = {}      # (kt, gb2) -> [128, 2048] tile (two batches)
            kts = {}      # (set, 0=e/1=d) -> [128, C]
            w32te = wpool.tile([128, 576], f16, tag="w32e")
            w32td = wpool.tile([128, 576], f16, tag="w32d")

            def load_quad_inputs(gb, first):
                # First quad's inputs alternate across both HWDGE rings
                # (sync + scalar): the ~0.63us per dma_start issue cost would
                # otherwise serialize the early input stream. The second quad
                # stays on sync -- scalar must be free for PSUM drains by
                # ~13us, and quad 1 is not latency-critical.
                ieng = [nc.sync, nc.scalar] if first else [nc.sync]
                iq = [0]

                def idma(dst, src):
                    ieng[iq[0] % len(ieng)].dma_start(dst, src)
                    iq[0] += 1

                for n in range(2):
                    for ki in range(2):
                        if first and n == 0:
                            t_ = wpool.tile([128, 576], f16, tag=f"we{ki}")
                            idma(t_[:], wte[128 * ki:128 * ki + 128, :])
                            wet[ki] = t_
                        for gb2 in (gb, gb + 2):
                            tx = xpool.tile([128, C], f16,
                                            tag=f"x{ki}_{n}_{gb2}")
                            idma(tx[:], eh[ki, :, n, gb2:gb2 + 2, :])
                            xts[(ki, n, gb2)] = tx
                    if n == 0:
                        if first:
                            idma(w32te[:], w32e[:, :])
                        tk = xpool.tile([128, C], f16, tag=f"kre{gb}")
                        idma(tk[:], krem[gb // 4, 0, :, :])
                        kts[(gb // 4, 0)] = tk
                    for ki in range(2):
                        if first and n == 0:
                            t_ = wpool.tile([128, 576], f16, tag=f"wd{ki}")
                            idma(t_[:], wtd[128 * ki:128 * ki + 128, :])
                            wdt[ki] = t_
                        for gb2 in (gb, gb + 2):
                            tx = xpool.tile([128, C], f16,
                                            tag=f"x{2 + ki}_{n}_{gb2}")
                            idma(tx[:], eh[2 + ki, :, n, gb2:gb2 + 2, :])
                            xts[(2 + ki, n, gb2)] = tx
                    if n == 0:
                        if first:
                            idma(w32td[:], w32d[:, :])
                        tk = xpool.tile([128, C], f16, tag=f"krd{gb}")
                        idma(tk[:], krem[gb // 4, 1, :, :])
                        kts[(gb // 4, 1)] = tk

            for gb in (0, 4):
                load_quad_inputs(gb, first=(gb == 0))

            wtile = {"e": wet, "d": wdt}
            w32tile = {"e": w32te, "d": w32td}

            def mov(op, ki, b, n):
                kt = (0 if op == "e" else 2) + ki
                gb2 = (b // 2) * 2
                c0 = 512 * (b - gb2)
                return xts[(kt, n, gb2)][:, c0:c0 + 512]

            for gi, (gb, n) in enumerate(groups):
                for mi in range(MT):
                    pts = [ps.tile([128, 512], f32, tag="pt", name="pt")
                           for _ in range(4)]
                    subs = msubs(mi)
                    for ki in range(2):
                        if mi == 4:
                            for bi in range(4):
                                for (op, w0, nc_, p0) in subs:
                                    nc.tensor.matmul(
                                        pts[bi][p0:p0 + nc_, :],
                                        wtile[op][ki][:, w0:w0 + nc_],
                                        mov(op, ki, gb + bi, n),
                                        start=(ki == 0), stop=False,
                                        tile_position=(0, p0),
                                    )
                        else:
                            op, w0, nc_, p0 = subs[0]
                            for bi in range(4):
                                nc.tensor.matmul(
                                    pts[bi][:],
                                    wtile[op][ki][:, w0:w0 + nc_],
                                    mov(op, ki, gb + bi, n),
                                    start=(ki == 0), stop=False,
                                )
                    # 32-row k-remainder last: 4 concurrent strip matmuls
                    # (full 128-row usage), one span per m-tile.
                    for (op, w0, nc_, p0) in subs:
                        kt = kts[(gb // 4, 0 if op == "e" else 1)]
                        for bi in range(4):
                            sp = 32 * bi
                            nc.tensor.matmul(
                                pts[bi][p0:p0 + nc_, :],
                                w32tile[op][sp:sp + 32, w0:w0 + nc_],
                                kt[sp:sp + 32, 512 * n:512 * n + 512],
                                start=False, stop=True,
                                tile_position=(sp, p0),
                            )
                    # drain psum -> sbuf (vector: b0/b1, scalar: b2/b3),
                    # one 4-batch DMA out. Early groups' outputs go on the
                    # SYNC ring: engine instruction order puts them behind
                    # all input dma_starts, so the input stream keeps the
                    # full early HBM bandwidth (opool absorbs the deferral).
                    # Late groups alternate both rings to drain the tail.
                    ot = opool.tile([128, 2 * C], f16, tag="ot")
                    nc.vector.tensor_copy(ot[:, 0:512], pts[0][:])
                    nc.vector.tensor_copy(ot[:, 512:1024], pts[1][:])
                    nc.scalar.copy(ot[:, 1024:1536], pts[2][:])
                    nc.scalar.copy(ot[:, 1536:2048], pts[3][:])
                    if gi == 3 and mi == MT - 1:
                        # final tile: split across both rings to cut the tail
                        nc.sync.dma_start(out[mi, :, n, gb:gb + 2, :],
                                          ot[:, 0:C])
                        nc.gpsimd.dma_start(out[mi, :, n, gb + 2:gb + 4, :],
                                            ot[:, C:2 * C])
                    else:
                        oeng = nc.gpsimd if (gi < 2 or mi % 2 == 1) else nc.sync
                        oeng.dma_start(out[mi, :, n, gb:gb + 4, :], ot[:])
    nc.finalize()
    return nc


def _get_nc():
    if "nc" not in _CACHED:
        _CACHED["nc"] = _build_nc()
    return _CACHED["nc"]


def _ensure_trace_hook_safe():
    """If BASS_TRACE is set, run_bass_kernel_spmd imports antenv.axon_hooks,
    which may not exist. Install a ctypes shim or disable tracing."""
    import os
    import sys
    import types

    if not os.environ.get("BASS_TRACE"):
        return
    try:
        import antenv.axon_hooks  # noqa: F401
        return
    except ImportError:
        pass
    try:
        from trn_agent_boot.trn_boot import _ntff_profile_via_ctypes
        hooks = types.ModuleType("antenv.axon_hooks")
        hook = _ntff_profile_via_ctypes("/opt/axon/libaxon_pjrt.so")
        hooks.get_axon_ntff_profile_hook = lambda: hook
        hooks.set_axon_ntff_profile_hook = lambda h: None
        sys.modules["antenv.axon_hooks"] = hooks
    except Exception:
        os.environ["BASS_NEVER_TRACE"] = "1"


def kernel(x: np.ndarray):
    from concourse.bass_utils import run_bass_kernel_spmd

    _ensure_trace_hook_safe()
    x = np.ascontiguousarray(np.asarray(x, dtype=np.float32))
    assert x.shape == (B, T, C)

    # ---- host: data-dependent truncation length L (tiny, exact math) ----
    M64 = _dct_mat(T)
    xbar = x.astype(np.float64).mean(axis=(0, 2))
    v = np.abs(M64 @ xbar)
    thr = np.abs(np.quantile(v, Q8))
    idxs = np.where(v > thr)[0]
    last_index = int(idxs[-1]) if idxs.size > 0 else -1
    L = last_index if last_index >= 0 else T - 1
    h = (L + 1) // 2

    # ---- host: fold inputs ----
    u = x[:, 0:H, :]
    w_ = x[:, T - 1:H - 1:-1, :]
    e = (u + w_).astype(np.float16)            # [B, 288, C]
    d = (u - w_).astype(np.float16)

    # [4, 128, 2, B, 512]: kt 0,1 = e k-tiles; 2,3 = d k-tiles; dim2 = n-half
    e_k = e[:, 0:256].reshape(B, 2, 128, 2, 512).transpose(1, 2, 3, 0, 4)
    d_k = d[:, 0:256].reshape(B, 2, 128, 2, 512).transpose(1, 2, 3, 0, 4)
    ehall = np.concatenate([e_k, d_k], axis=0)
    # [B//4, 2, 128, C]: 4 batches' 32-row remainders stacked per set
    kre = e[:, 256:288].reshape(B // 4, 4 * 32, C)
    krd = d[:, 256:288].reshape(B // 4, 4 * 32, C)
    krall = np.stack([kre, krd], axis=1)

    # ---- host: weights ----
    key = ("w", L)
    if key not in _CACHED:
        Me = M64[0::2, 0:H]
        Mo = M64[1::2, 0:H]
        Mi = _dct_mat(L)
        Ae = Mi[0::2, 0:h].T @ M64[0:L:2, 0:H]      # [h, 288]
        Ao = Mi[1::2, 0:h].T @ M64[1:L:2, 0:H]      # [h, 288]
        wte_np = np.zeros((H, 576), dtype=np.float16)
        wtd_np = np.zeros((H, 576), dtype=np.float16)
        wte_np[:, 0:H] = Me.T
        wte_np[:, H:H + h] = Ae.T
        wtd_np[:, 0:H] = Mo.T
        wtd_np[:, H:H + h] = Ao.T
        w32e_np = np.ascontiguousarray(np.tile(wte_np[256:288], (4, 1)))
        w32d_np = np.ascontiguousarray(np.tile(wtd_np[256:288], (4, 1)))
        _CACHED[key] = (wte_np, wtd_np, w32e_np, w32d_np)
    wte_np, wtd_np, w32e_np, w32d_np = _CACHED[key]

    nc = _get_nc()
    in_maps = [
        {"eh": np.ascontiguousarray(ehall[:, :, :, i * BPC:(i + 1) * BPC]),
         "krem": np.ascontiguousarray(
             krall[i * BPC // 4:(i + 1) * BPC // 4]),
         "wte": wte_np, "wtd": wtd_np, "w32e": w32e_np, "w32d": w32d_np}
        for i in range(NCORES)
    ]
    res = run_bass_kernel_spmd(nc, in_maps, list(range(NCORES)))
    _CACHED["last_exec_time_ns"] = res.exec_time_ns
    _CACHED["profile_json"] = res.profile_json

    # out [MT, 128, 2, BPC, 512] per core -> [BPC, 1152, C]
    o = np.concatenate(
        [res.results[i]["out"].transpose(3, 0, 1, 2, 4).reshape(
            BPC, MT * 128, C)
         for i in range(NCORES)], axis=0)
    Xe = o[:, 0:288]
    P = o[:, 288:576]
    Xo = o[:, 576:864]
    Qm = o[:, 864:1152]

    n_even = (L + 1) // 2
    n_odd = L // 2
    y = np.empty((B, L, C), dtype=np.float32)
    y[:, 0::2] = Xe[:, :n_even]
    y[:, 1::2] = Xo[:, :n_odd]

    Pf = P[:, :h].astype(np.float32)
    Qf = Qm[:, :h].astype(np.float32)
    state = np.empty((B, L, C), dtype=np.float16)
    state[:, 0:h] = (Pf + Qf).astype(np.float16)
    state[:, h:L] = (Pf - Qf)[:, 0:L - h][:, ::-1].astype(np.float16)
    return state, y


# revision 41
# speedup vs baseline: 1.0539x; 1.0076x over previous
"""Trainium2 Bass kernel for DCTLAVISBlip dc_transform (DCT -> truncate -> IDCT).

Measured 102.9-105.8 us on hardware across runs (run-to-run HW variance
~+-1.5 us; baseline dense version: ~170.8 us) at relative error ~7e-4 (gate
2e-2). Breakdown: ~70 us matmul-span floor (324 spans x ~216 ns), ~8 us
LDWEIGHTS bubbles + ~3 us input-catchup gaps (ring-bandwidth bound), ~17 us
fixed NEFF preamble/teardown (engine init, all-engine barriers, full
semaphore-file clear), ~5 us drain/DMA tails. DMA 29.1 MB/core, peak ~430
GB/s combined R+W.

Strategy (exact even/odd DCT factorization, half the matmul FLOPs)
------------------------------------------------------------------
reference(x), x [B=64, T=576, C=1024] f32:
  y = M @ x[b] (DCT along tokens), v = |mean_{b,c} y|, threshold = quantile(v, .8),
  L = last index with v>thr; outputs y[:, :L] (f32) and state = Mi^T @ y[:, :L] (f16).

The DCT matrix obeys M[2j, T-1-t] = M[2j, t] and M[2j+1, T-1-t] = -M[2j+1, t].
With e = x_top + reverse(x_bot), d = x_top - reverse(x_bot)  ([288, C] each):
  y[0::2] = Me @ e,   y[1::2] = Mo @ d          (Me = M[0::2, :288], Mo = M[1::2, :288])
The same symmetry on Mi (size L) splits the IDCT: with h = ceil(L/2),
  P = Ae @ e, Q = Ao @ d   (Ae = Mi[0::2, :h]^T @ M[0:L:2, :288], Ao likewise odd)
  state[0:h] = P + Q,  state[h:L] = reverse((P - Q)[0:L-h])
All folds/reassembly are cheap O(B*T*C) host ops; the device does the four
[<=288 x 288] @ [288 x 1024] matmul blocks per batch -- exactly half the dense
[1152 x 576] work of v1 -- as one stacked 9-m-tile output per batch,
data-parallel over B across 8 cores.

Device schedule per core (8 batches): groups of (4 batches x 1 n-half) share
4 PSUM banks per m-tile, so the 32-row K-remainder row-packs 4-up (4 batches)
into one full-height 128-partition tile and runs as 4 concurrent strip
matmuls (tile_position) -- one span instead of two. m-tile 4 mixes the last
64 e-rows and first 64 d-rows via col-split matmuls (tile_position col
groups). DRAM tensors are laid out so one DMA moves a whole group's slab with
4KB-contiguous output rows; input k-tiles are n-half-split 256KB slabs so the
first matmuls aren't gated on large transfers. The first quad's inputs
alternate across both HWDGE rings (sync + scalar) to beat the ~0.63us
per-dma_start issue cost; outputs go on gpsimd SWDGE plus the sync ring late.
PSUM drains split vector/scalar; PE pre-warmed with wide dummy matmuls that
start as soon as the Tensor engine is up.
"""

import numpy as np

B, T, C = 64, 576, 1024
H = T // 2                   # 288, fold length
NCORES = 8
BPC = B // NCORES            # batches per core
MT = 9                       # m-tiles over 1152 output rows
Q8 = 0.8

_CACHED = {}


def _dct_mat(N):
    n = np.arange(N)
    Mm = np.cos(np.pi * (2 * n[None, :] + 1) * n[:, None] / (2 * N))
    s = np.full(N, np.sqrt(2.0 / N))
    s[0] = np.sqrt(1.0 / N)
    return s[:, None] * Mm          # float64


def _build_nc():
    import concourse.bacc as bacc
    import concourse.mybir as mybir
    import concourse.tile as tile

    f16 = mybir.dt.float16
    f32 = mybir.dt.float32

    nc = bacc.Bacc("TRN2", target_bir_lowering=False, debug=False,
                   num_devices=NCORES)
    # k-tiles laid out [kt, 128, nhalf, batch, 512]: kt 0,1 = e rows 0:256;
    # 2,3 = d rows. 2-batch 256KB DMA granularity: fine enough that the
    # first matmuls aren't gated on whole-quad transfers, coarse enough to
    # keep the per-dma_start issue cost manageable.
    eh = nc.dram_tensor("eh", [4, 128, 2, BPC, 512], f16, kind="ExternalInput")
    # k-remainder strips: [set, e/d, 128, C]; rows = 4 batches x 32
    krem = nc.dram_tensor("krem", [BPC // 4, 2, 128, C], f16,
                          kind="ExternalInput")
    # weights [k, m]: wte m-cols = [Me.T (288) | Ae.T (h, pad->288)]
    wte = nc.dram_tensor("wte", [H, 576], f16, kind="ExternalInput")
    wtd = nc.dram_tensor("wtd", [H, 576], f16, kind="ExternalInput")
    # k-remainder weights, duplicated across the 4 row strips
    w32e = nc.dram_tensor("w32e", [128, 576], f16, kind="ExternalInput")
    w32d = nc.dram_tensor("w32d", [128, 576], f16, kind="ExternalInput")
    # out rows (over m-tiles): [Xe(288); P(288); Xo(288); Q(288)]
    out = nc.dram_tensor("out", [MT, 128, 2, BPC, 512], f16,
                         kind="ExternalOutput")

    # groups: (batch-quad, n-half)
    groups = [(gb, n) for gb in (0, 4) for n in (0, 1)]

    # m-tile sub-blocks: (operand, wcol0, ncols, psum_col0)
    def msubs(mi):
        if mi <= 3:
            return [("e", 128 * mi, 128, 0)]
        if mi == 4:
            return [("e", 512, 64, 0), ("d", 0, 64, 64)]
        return [("d", 64 + 128 * (mi - 5), 128, 0)]

    with tile.TileContext(nc) as tc:
        with (
            tc.tile_pool(name="wpool", bufs=1) as wpool,
            tc.tile_pool(name="xpool", bufs=1) as xpool,
            tc.tile_pool(name="osb", bufs=16) as opool,
            tc.tile_pool(name="ps", bufs=8, space="PSUM") as ps,
        ):
            # PE warmup during the input-DMA head: wide (N=512) matmuls keep
            # the PE busy-duty high enough to trip the HAM un-throttle before
            # the first real matmul, and run until the first inputs land.
            # The memset comes AFTER the matmuls in program order: the tile
            # allocator just needs wz written somewhere, while the matmuls
            # (garbage reads, results discarded) start as soon as the Tensor
            # engine is up (~3.4us) instead of waiting for gpsimd init.
            wz = wpool.tile([128, 512], f16, tag="wz")
            pwarm = ps.tile([128, 512], f32, tag="pt", name="pt")
            for _ in range(5):
                nc.tensor.matmul(pwarm[:], wz[:, 0:128], wz[:],
                                 start=True, stop=True)
            nc.gpsimd.memset(wz[:], 0.0)

            # ---- input DMAs in first-use order (sync HWDGE ring) ----
            wet = [None, None]
            wdt = [None, None]
            xts = {}      # (kt, gb2) -> [128, 2048] tile (two batches)
            kts = {}      # (set, 0=e/1=d) -> [128, C]
            w32te = wpool.tile([128, 576], f16, tag="w32e")
            w32td = wpool.tile([128, 576], f16, tag="w32d")

            def load_quad_inputs(gb, first):
                # First quad's inputs alternate across both HWDGE rings
                # (sync + scalar): the ~0.63us per dma_start issue cost would
                # otherwise serialize the early input stream. The second quad
                # stays on sync -- scalar must be free for PSUM drains by
                # ~13us, and quad 1 is not latency-critical.
                ieng = [nc.sync, nc.scalar] if first else [nc.sync]
                iq = [0]

                def idma(dst, src):
                    ieng[iq[0] % len(ieng)].dma_start(dst, src)
                    iq[0] += 1

                for n in range(2):
                    for ki in range(2):
                        if first and n == 0:
                            t_ = wpool.tile([128, 576], f16, tag=f"we{ki}")
                            idma(t_[:], wte[128 * ki:128 * ki + 128, :])
                            wet[ki] = t_
                        for gb2 in (gb, gb + 2):
                            tx = xpool.tile([128, C], f16,
                                            tag=f"x{ki}_{n}_{gb2}")
                            idma(tx[:], eh[ki, :, n, gb2:gb2 + 2, :])
                            xts[(ki, n, gb2)] = tx
                    if n == 0:
                        if first:
                            idma(w32te[:], w32e[:, :])
                        tk = xpool.tile([128, C], f16, tag=f"kre{gb}")
                        idma(tk[:], krem[gb // 4, 0, :, :])
                        kts[(gb // 4, 0)] = tk
                    for ki in range(2):
                        if first and n == 0:
                            t_ = wpool.tile([128, 576], f16, tag=f"wd{ki}")
                            idma(t_[:], wtd[128 * ki:128 * ki + 128, :])
                            wdt[ki] = t_
                        for gb2 in (gb, gb + 2):
                            tx = xpool.tile([128, C], f16,
                                            tag=f"x{2 + ki}_{n}_{gb2}")
                            idma(tx[:], eh[2 + ki, :, n, gb2:gb2 + 2, :])
                            xts[(2 + ki, n, gb2)] = tx
                    if n == 0:
                        if first:
                            idma(w32td[:], w32d[:, :])
                        tk = xpool.tile([128, C], f16, tag=f"krd{gb}")
                        idma(tk[:], krem[gb // 4, 1, :, :])
                        kts[(gb // 4, 1)] = tk

            for gb in (0, 4):
                load_quad_inputs(gb, first=(gb == 0))

            wtile = {"e": wet, "d": wdt}
            w32tile = {"e": w32te, "d": w32td}

            def mov(op, ki, b, n):
                kt = (0 if op == "e" else 2) + ki
                gb2 = (b // 2) * 2
                c0 = 512 * (b - gb2)
                return xts[(kt, n, gb2)][:, c0:c0 + 512]

            for gi, (gb, n) in enumerate(groups):
                for mi in range(MT):
                    pts = [ps.tile([128, 512], f32, tag="pt", name="pt")
                           for _ in range(4)]
                    subs = msubs(mi)
                    for ki in range(2):
                        if mi == 4:
                            for bi in range(4):
                                for (op, w0, nc_, p0) in subs:
                                    nc.tensor.matmul(
                                        pts[bi][p0:p0 + nc_, :],
                                        wtile[op][ki][:, w0:w0 + nc_],
                                        mov(op, ki, gb + bi, n),
                                        start=(ki == 0), stop=False,
                                        tile_position=(0, p0),
                                    )
                        else:
                            op, w0, nc_, p0 = subs[0]
                            for bi in range(4):
                                nc.tensor.matmul(
                                    pts[bi][:],
                                    wtile[op][ki][:, w0:w0 + nc_],
                                    mov(op, ki, gb + bi, n),
                                    start=(ki == 0), stop=False,
                                )
                    # 32-row k-remainder last: 4 concurrent strip matmuls
                    # (full 128-row usage), one span per m-tile.
                    for (op, w0, nc_, p0) in subs:
                        kt = kts[(gb // 4, 0 if op == "e" else 1)]
                        for bi in range(4):
                            sp = 32 * bi
                            nc.tensor.matmul(
                                pts[bi][p0:p0 + nc_, :],
                                w32tile[op][sp:sp + 32, w0:w0 + nc_],
                                kt[sp:sp + 32, 512 * n:512 * n + 512],
                                start=False, stop=True,
                                tile_position=(sp, p0),
                            )
                    # drain psum -> sbuf (vector: b0/b1, scalar: b2/b3),
                    # one 4-batch DMA out. Early groups' outputs go on the
                    # SYNC ring: engine instruction order puts them behind
                    # all input dma_starts, so the input stream keeps the
                    # full early HBM bandwidth (opool absorbs the deferral).
                    # Late groups alternate both rings to drain the tail.
                    ot = opool.tile([128, 2 * C], f16, tag="ot")
                    nc.vector.tensor_copy(ot[:, 0:512], pts[0][:])
                    nc.vector.tensor_copy(ot[:, 512:1024], pts[1][:])
                    nc.scalar.copy(ot[:, 1024:1536], pts[2][:])
                    nc.scalar.copy(ot[:, 1536:2048], pts[3][:])
                    if gi == 3 and mi == MT - 1:
                        # final tile: split across both rings to cut the tail
                        nc.sync.dma_start(out[mi, :, n, gb:gb + 2, :],
                                          ot[:, 0:C])
                        nc.gpsimd.dma_start(out[mi, :, n, gb + 2:gb + 4, :],
                                            ot[:, C:2 * C])
                    else:
                        oeng = nc.gpsimd if (gi < 2 or mi % 2 == 1) else nc.sync
                        oeng.dma_start(out[mi, :, n, gb:gb + 4, :], ot[:])
    nc.finalize()
    return nc


def _get_nc():
    if "nc" not in _CACHED:
        _CACHED["nc"] = _build_nc()
    return _CACHED["nc"]


def _ensure_trace_hook_safe():
    """If BASS_TRACE is set, run_bass_kernel_spmd imports antenv.axon_hooks,
    which may not exist. Install a ctypes shim or disable tracing."""
    import os
    import sys
    import types

    if not os.environ.get("BASS_TRACE"):
        return
    try:
        import antenv.axon_hooks  # noqa: F401
        return
    except ImportError:
        pass
    try:
        from trn_agent_boot.trn_boot import _ntff_profile_via_ctypes
        hooks = types.ModuleType("antenv.axon_hooks")
        hook = _ntff_profile_via_ctypes("/opt/axon/libaxon_pjrt.so")
        hooks.get_axon_ntff_profile_hook = lambda: hook
        hooks.set_axon_ntff_profile_hook = lambda h: None
        sys.modules["antenv.axon_hooks"] = hooks
    except Exception:
        os.environ["BASS_NEVER_TRACE"] = "1"


def kernel(x: np.ndarray):
    from concourse.bass_utils import run_bass_kernel_spmd

    _ensure_trace_hook_safe()
    x = np.ascontiguousarray(np.asarray(x, dtype=np.float32))
    assert x.shape == (B, T, C)

    # ---- host: data-dependent truncation length L (tiny, exact math) ----
    M64 = _dct_mat(T)
    xbar = x.astype(np.float64).mean(axis=(0, 2))
    v = np.abs(M64 @ xbar)
    thr = np.abs(np.quantile(v, Q8))
    idxs = np.where(v > thr)[0]
    last_index = int(idxs[-1]) if idxs.size > 0 else -1
    L = last_index if last_index >= 0 else T - 1
    h = (L + 1) // 2

    # ---- host: fold inputs ----
    u = x[:, 0:H, :]
    w_ = x[:, T - 1:H - 1:-1, :]
    e = (u + w_).astype(np.float16)            # [B, 288, C]
    d = (u - w_).astype(np.float16)

    # [4, 128, 2, B, 512]: kt 0,1 = e k-tiles; 2,3 = d k-tiles; dim2 = n-half
    e_k = e[:, 0:256].reshape(B, 2, 128, 2, 512).transpose(1, 2, 3, 0, 4)
    d_k = d[:, 0:256].reshape(B, 2, 128, 2, 512).transpose(1, 2, 3, 0, 4)
    ehall = np.concatenate([e_k, d_k], axis=0)
    # [B//4, 2, 128, C]: 4 batches' 32-row remainders stacked per set
    kre = e[:, 256:288].reshape(B // 4, 4 * 32, C)
    krd = d[:, 256:288].reshape(B // 4, 4 * 32, C)
    krall = np.stack([kre, krd], axis=1)

    # ---- host: weights ----
    key = ("w", L)
    if key not in _CACHED:
        Me = M64[0::2, 0:H]
        Mo = M64[1::2, 0:H]
        Mi = _dct_mat(L)
        Ae = Mi[0::2, 0:h].T @ M64[0:L:2, 0:H]      # [h, 288]
        Ao = Mi[1::2, 0:h].T @ M64[1:L:2, 0:H]      # [h, 288]
        wte_np = np.zeros((H, 576), dtype=np.float16)
        wtd_np = np.zeros((H, 576), dtype=np.float16)
        wte_np[:, 0:H] = Me.T
        wte_np[:, H:H + h] = Ae.T
        wtd_np[:, 0:H] = Mo.T
        wtd_np[:, H:H + h] = Ao.T
        w32e_np = np.ascontiguousarray(np.tile(wte_np[256:288], (4, 1)))
        w32d_np = np.ascontiguousarray(np.tile(wtd_np[256:288], (4, 1)))
        _CACHED[key] = (wte_np, wtd_np, w32e_np, w32d_np)
    wte_np, wtd_np, w32e_np, w32d_np = _CACHED[key]

    nc = _get_nc()
    in_maps = [
        {"eh": np.ascontiguousarray(ehall[:, :, :, i * BPC:(i + 1) * BPC]),
         "krem": np.ascontiguousarray(
             krall[i * BPC // 4:(i + 1) * BPC // 4]),
         "wte": wte_np, "wtd": wtd_np, "w32e": w32e_np, "w32d": w32d_np}
        for i in range(NCORES)
    ]
    res = run_bass_kernel_spmd(nc, in_maps, list(range(NCORES)))
    _CACHED["last_exec_time_ns"] = res.exec_time_ns
    _CACHED["profile_json"] = res.profile_json

    # out [MT, 128, 2, BPC, 512] per core -> [BPC, 1152, C]
    o = np.concatenate(
        [res.results[i]["out"].transpose(3, 0, 1, 2, 4).reshape(
            BPC, MT * 128, C)
         for i in range(NCORES)], axis=0)
    Xe = o[:, 0:288]
    P = o[:, 288:576]
    Xo = o[:, 576:864]
    Qm = o[:, 864:1152]

    n_even = (L + 1) // 2
    n_odd = L // 2
    y = np.empty((B, L, C), dtype=np.float32)
    y[:, 0::2] = Xe[:, :n_even]
    y[:, 1::2] = Xo[:, :n_odd]

    Pf = P[:, :h].astype(np.float32)
    Qf = Qm[:, :h].astype(np.float32)
    state = np.empty((B, L, C), dtype=np.float16)
    state[:, 0:h] = (Pf + Qf).astype(np.float16)
    state[:, h:L] = (Pf - Qf)[:, 0:L - h][:, ::-1].astype(np.float16)
    return state, y
